# Initial kernel scaffold
#
"""Multi-head causal attention (LLaMA RoPE), head-parallel sharding on 8 trn2 cores.

Core c = (batch b=c//2, head-group g=c%2). Each core computes Q/K/V projections
and attention for its 8 heads over the FULL 1024 rows of its batch (zero
duplication), writes its attention output (ot, [8*128 d, 1024 q]) to DRAM, and
a pair AllGather ([0,1],[2,3],...) exchanges ot between the two head-groups of
each batch. Each core then computes the output projection for its e-column
half (full 2048-dim contraction) -> y [1024, 1024].

QKV/attention matmuls fp32r; the gathered ot and wo run bf16 (halves exchange
wire time + wo DMA; ~0.3% rms, well inside the 2e-2 gate). Scores are computed
transposed ST[k,q]; softmax skips max-subtraction; denominator via ones-matmul
accumulated alongside PV; causal mask applied post-exp as a binary multiply on
diagonal 128x128 tiles (exact for the additive -1e9/0 causal mask).
"""

import math
import sys

import numpy as np

sys.path.insert(0, "/opt/trn_rl_repo")

B, S, DIM, H = 4, 1024, 2048, 16
HD = DIM // H  # 128
KC = DIM // 128  # 16 contraction chunks
HPC = H // 2  # 8 heads per core
NP = HPC // 2  # 4 head-pairs per core
EHALF = DIM // 2  # 1024 output cols per core
SCALE = 1.0 / math.sqrt(HD)
N_CORES = 8
PAIRS = [[0, 1], [2, 3], [4, 5], [6, 7]]
# O-proj accumulation order: pair-3 head chunks last (its AllGather lands last)
DC_ORDER = [0, 1, 8, 9, 2, 3, 10, 11, 4, 5, 12, 13, 6, 7, 14, 15]

_cache = {}


def _build_nc():
    import concourse.mybir as mybir
    import concourse.tile as tile
    from concourse import bacc

    F32R = mybir.dt.float32r
    F32 = mybir.dt.float32
    BF16 = mybir.dt.bfloat16

    nc = bacc.Bacc("TRN2", target_bir_lowering=False, debug=False,
                   num_devices=N_CORES)

    x_in = nc.dram_tensor("x_pre", [2, 128, KC, 512], F32R, kind="ExternalInput")
    wq_in = nc.dram_tensor("wq_pre", [HPC, 128, KC, 128], F32R, kind="ExternalInput")
    wk_in = nc.dram_tensor("wk_pre", [HPC, 128, KC, 128], F32R, kind="ExternalInput")
    wv_in = nc.dram_tensor("wv_pre", [NP, 128, KC, 256], F32R, kind="ExternalInput")
    wo_in = nc.dram_tensor("wo_pre", [NP, 128, 4, EHALF], BF16, kind="ExternalInput")
    bq_in = nc.dram_tensor("bq_p", [128, HPC, 1], F32, kind="ExternalInput")
    bk_in = nc.dram_tensor("bk_p", [128, HPC, 1], F32, kind="ExternalInput")
    bv_in = nc.dram_tensor("bv_p", [128, NP, 256], F32, kind="ExternalInput")
    csk_in = nc.dram_tensor("csk2", [128, S], F32R, kind="ExternalInput")
    ssk_in = nc.dram_tensor("ssk2", [128, S], F32R, kind="ExternalInput")
    triw_in = nc.dram_tensor("triw", [128, 256], F32R, kind="ExternalInput")
    ones_in = nc.dram_tensor("ones128", [128, 128], F32R, kind="ExternalInput")
    y_out = nc.dram_tensor("y", [S, EHALF], F32, kind="ExternalOutput")

    with tile.TileContext(nc) as tc:
        with (
            tc.tile_pool(name="consts", bufs=1) as consts,
            tc.tile_pool(name="xpool", bufs=1) as xpool,
            tc.tile_pool(name="trig", bufs=1) as trig,
            tc.tile_pool(name="wpool", bufs=1) as wpool,
            tc.tile_pool(name="qkv", bufs=1) as qkv,
            tc.tile_pool(name="rope", bufs=3) as rope,
            tc.tile_pool(name="ptp", bufs=2) as ptp,
            tc.tile_pool(name="otst", bufs=2) as otst,
            tc.tile_pool(name="sotp", bufs=3) as sotp,
            tc.tile_pool(name="yap", bufs=1) as yap,
            tc.tile_pool(name="dram", bufs=1, space="DRAM") as dram,
            tc.tile_pool(name="ps_qk", bufs=2, space="PSUM") as ps_qk,
            tc.tile_pool(name="ps_v", bufs=1, space="PSUM") as ps_v,
            tc.tile_pool(name="ps_st", bufs=2, space="PSUM") as ps_st,
            tc.tile_pool(name="ps_acc", bufs=3, space="PSUM") as ps_acc,
        ):
            def load_pair_weights(p, x_interleave=None):
                """Allocate + DMA weight tiles for pair p. x_interleave: list of
                (sbuf_dst, dram_src) x-chunk DMAs dripped between weight DMAs
                so the first Q-projection's deps land early."""
                xi = list(x_interleave or [])

                def drip(n):
                    for _ in range(n):
                        if xi:
                            dst, src = xi.pop(0)
                            nc.sync.dma_start(dst, src)

                wq_sb = [wpool.tile([128, KC, 128], F32R, tag=f"wq{lh}",
                                    name="wq_sb") for lh in range(2)]
                wk_sb = [wpool.tile([128, KC, 128], F32R, tag=f"wk{lh}",
                                    name="wk_sb") for lh in range(2)]
                wv_sb = wpool.tile([128, KC, 256], F32R, tag="wv", name="wv_sb")
                nc.sync.dma_start(wq_sb[0][:], wq_in[2 * p])
                drip(8)
                nc.sync.dma_start(wq_sb[1][:], wq_in[2 * p + 1])
                nc.sync.dma_start(wk_sb[0][:], wk_in[2 * p])
                nc.sync.dma_start(wk_sb[1][:], wk_in[2 * p + 1])
                nc.sync.dma_start(wv_sb[:], wv_in[p])
                return wq_sb, wk_sb, wv_sb

            def load_wo(p):
                wo_sb = wpool.tile([128, 4, EHALF], BF16, tag="wo", bufs=2,
                                   name="wo_sb")
                nc.sync.dma_start(wo_sb[:], wo_in[p])
                return wo_sb

            # small rope tables + biases first (needed ~20us in)
            bq_sb = consts.tile([128, HPC, 1], F32, tag="bq")
            bk_sb = consts.tile([128, HPC, 1], F32, tag="bk")
            nc.sync.dma_start(bq_sb[:], bq_in[:])
            nc.sync.dma_start(bk_sb[:], bk_in[:])
            # pair-0 weights interleaved with x chunks so the first
            # Q-projection (wq0 + x) can start early
            x_sb = xpool.tile([128, KC, S], F32R, tag="x", name="x_sb")
            x_chunks = [(x_sb[:, 4 * i:4 * i + 4, h * 512:h * 512 + 512],
                         x_in[h][:, 4 * i:4 * i + 4, :])
                        for h in range(2) for i in range(4)]
            w0 = load_pair_weights(0, x_interleave=x_chunks)

            csk_sb = trig.tile([128, S], F32R, tag="cs", name="csk_sb")
            ssk_sb = trig.tile([128, S], F32R, tag="ss", name="ssk_sb")
            nc.sync.dma_start(csk_sb[:], csk_in[:])
            nc.sync.dma_start(ssk_sb[:], ssk_in[:])

            bv_sb = consts.tile([128, NP, 256], F32, tag="bv")
            nc.sync.dma_start(bv_sb[:], bv_in[:])
            triw_sb = consts.tile([128, 256], F32R)
            nc.sync.dma_start(triw_sb[:], triw_in[:])
            ones_sb = consts.tile([128, 128], F32R)
            nc.sync.dma_start(ones_sb[:], ones_in[:])

            y_acc = yap.tile([128, 8, 2, 512], BF16, name="y_acc")
            wo_tiles = {}
            ag_ins = []
            ag_outs = []
            for p in range(NP - 1):
                ag_ins.append(dram.tile([256, S], BF16, name=f"agi{p}"))
                ag_outs.append(dram.tile([4, 128, S], BF16, name=f"ago{p}"))
            ag3_ins = [dram.tile([256, 512], BF16, name=f"agi3q{qc}")
                       for qc in range(2)]
            ag3_outs = [dram.tile([4, 128, 512], BF16, name=f"ago3q{qc}")
                        for qc in range(2)]

            def rope_chunk(pm, b_ap, cs_ap, ss_ap, dst_ap):
                """dst = rope(pm + bias); all [128, 512]."""
                tmp = rope.tile([128, 512], F32R, tag="tmp", name="tmp")
                nc.scalar.activation(
                    tmp[:], pm, mybir.ActivationFunctionType.Identity, bias=b_ap
                )
                tsw = rope.tile([128, 512], F32R, tag="tsw", name="tsw")
                nc.sync.dma_start(tsw[0:64, :], tmp[64:128, :])
                nc.sync.dma_start(tsw[64:128, :], tmp[0:64, :])
                nc.vector.tensor_mul(tmp[:], tmp[:], cs_ap)
                nc.vector.tensor_mul(tsw[:], tsw[:], ss_ap)
                nc.vector.tensor_add(dst_ap, tmp[:], tsw[:])

            def o_pass(p, wo_p):
                """Accumulate pair p's 4 head-chunks into y (bf16 partials)."""
                for qt in range(8):
                    qcols = slice(qt * 128, qt * 128 + 128)
                    sot = sotp.tile([128, 4, 128], BF16, tag="sot", name="sot")
                    if p < 3:
                        src_ap = ag_outs[p].rearrange("a r c -> r a c")[:, :, qcols]
                    else:
                        src_ap = ag3_outs[qt // 4].rearrange("a r c -> r a c")[
                            :, :, (qt % 4) * 128:(qt % 4) * 128 + 128]
                    nc.sync.dma_start(sot[:], src_ap)
                    for eb in range(2):
                        ecols = slice(eb * 512, eb * 512 + 512)
                        y_ps = ps_qk.tile([128, 512], F32, tag="qk", name="y_ps")
                        for a in range(4):
                            nc.tensor.matmul(y_ps[:], sot[:, a, :],
                                             wo_p[:, a, ecols],
                                             start=(a == 0), stop=(a == 3))
                        ya = y_acc[:, qt, eb, :]
                        if p == 0:
                            nc.vector.tensor_copy(ya, y_ps[:])
                        elif p < 3:
                            nc.vector.tensor_add(ya, ya, y_ps[:])
                        else:
                            y_sb = otst.tile([128, 512], F32, tag="y",
                                             name="y_sb")
                            nc.vector.tensor_add(y_sb[:], ya, y_ps[:])
                            nc.sync.dma_start(y_out[qcols, ecols], y_sb[:])

            w_next = w0
            for p in range(NP):
                wq_sb, wk_sb, wv_sb = w_next
                wo_tiles[p] = load_wo(p)

                # ---- Q/K projections + RoPE ----
                q_sb = [qkv.tile([128, S], F32R, tag=f"q{lh}", name="q_sb")
                        for lh in range(2)]
                k_sb = [qkv.tile([128, S], F32R, tag=f"k{lh}", name="k_sb")
                        for lh in range(2)]
                for lh in range(2):
                    h = 2 * p + lh
                    projs = ((q_sb[lh], wq_sb[lh], bq_sb),
                             (k_sb[lh], wk_sb[lh], bk_sb))
                    # lh1: K before Q so attention-lh0's exps never queue
                    # behind the last projection's rope identities on Act
                    for dst, w_sb, b_sb in (projs if lh == 0 else projs[::-1]):
                        for c in range(2):
                            ccols = slice(c * 512, c * 512 + 512)
                            pm = ps_qk.tile([128, 512], F32, tag="qk", name="pm")
                            for kc in range(KC):
                                nc.tensor.matmul(pm[:], w_sb[:, kc, :],
                                                 x_sb[:, kc, ccols],
                                                 start=(kc == 0),
                                                 stop=(kc == KC - 1))
                            rope_chunk(pm[:], b_sb[:, h, :], csk_sb[:, ccols],
                                       ssk_sb[:, ccols], dst[:, ccols])

                # ---- V projection ----
                v_sb = qkv.tile([128, 8, 256], F32R, tag="v", name="v_sb")
                for st in range(8):
                    scols = slice(st * 128, st * 128 + 128)
                    vp = ps_v.tile([128, 256], F32, tag="v", name="vp")
                    for kc in range(KC):
                        nc.tensor.matmul(vp[:], x_sb[:, kc, scols],
                                         wv_sb[:, kc, :],
                                         start=(kc == 0), stop=(kc == KC - 1))
                    nc.vector.tensor_add(v_sb[:, st, :], vp[:], bv_sb[:, p, :])

                if p + 1 < NP:
                    w_next = load_pair_weights(p + 1)
                if p == 2:
                    o_pass(0, wo_tiles[0])

                # ---- attention, both heads ----
                # pair 3 runs qc-major so its exchange splits into two
                # half-AllGathers, the first issued mid-attention
                lq = [(lh, qc) for lh in range(2) for qc in range(2)] \
                    if p < 3 else [(lh, qc) for qc in range(2) for lh in range(2)]
                for lh, qc in lq:
                    if True:
                        vcols = slice(lh * 128, lh * 128 + 128)
                        kts = range(4) if qc == 0 else range(8)
                        l_ps = ps_acc.tile([128, 512], F32, tag="acc", name="l_ps")
                        o_ps = ps_acc.tile([128, 512], F32, tag="acc", name="o_ps")
                        for kt in kts:
                            if qc == 0:
                                qv = slice(kt * 128, 512)
                                dg = slice(kt * 128, kt * 128 + 128)
                            else:
                                qv = slice((kt - 4) * 128, 512) if kt >= 4 \
                                    else slice(0, 512)
                                dg = slice((kt - 4) * 128, (kt - 4) * 128 + 128) \
                                    if kt >= 4 else None
                            wide = qv.stop - qv.start == 128
                            if wide:
                                qv = slice(qv.start - 128, 512)
                                dg = qv
                            qg = slice(qc * 512 + qv.start, qc * 512 + 512)
                            st_ps = ps_st.tile([128, 512], F32, tag="st",
                                               name="st_ps")
                            nc.tensor.matmul(st_ps[:, qv],
                                             k_sb[lh][:, kt * 128:kt * 128 + 128],
                                             q_sb[lh][:, qg],
                                             start=True, stop=True)
                            pt = ptp.tile([128, 512], F32R, tag="pt", name="pt")
                            nc.scalar.activation(pt[:, qv], st_ps[:, qv],
                                                 mybir.ActivationFunctionType.Exp,
                                                 scale=SCALE)
                            if dg is not None:
                                msk = triw_sb[:] if wide \
                                    else triw_sb[:, 128:256]
                                nc.vector.tensor_mul(pt[:, dg], pt[:, dg], msk)
                            first, last = (kt == kts[0]), (kt == kts[-1])
                            nc.tensor.matmul(l_ps[:, qv], ones_sb[:], pt[:, qv],
                                             start=first, stop=last)
                            nc.tensor.matmul(o_ps[:, qv], v_sb[:, kt, vcols],
                                             pt[:, qv], start=first, stop=last)
                        rl = rope.tile([128, 512], F32, tag="tmp", name="rl")
                        nc.vector.reciprocal_approx_fast(rl[:], l_ps[:])
                        ots = otst.tile([128, 512], BF16, tag="ot", name="ots")
                        nc.vector.tensor_mul(ots[:], o_ps[:], rl[:])
                        if p < 3:
                            nc.sync.dma_start(
                                ag_ins[p][lh * 128:lh * 128 + 128,
                                          qc * 512:qc * 512 + 512], ots[:])
                        else:
                            nc.sync.dma_start(
                                ag3_ins[qc][lh * 128:lh * 128 + 128, :], ots[:])
                            if lh == 1:
                                nc.gpsimd.collective_compute(
                                    "AllGather", mybir.AluOpType.bypass,
                                    replica_groups=PAIRS,
                                    ins=[ag3_ins[qc].opt()],
                                    outs=[ag3_outs[qc].opt()],
                                )

                if p < 3:
                    nc.gpsimd.collective_compute(
                        "AllGather", mybir.AluOpType.bypass, replica_groups=PAIRS,
                        ins=[ag_ins[p].opt()], outs=[ag_outs[p].opt()],
                    )
                if p >= 2:
                    o_pass(p - 1, wo_tiles[p - 1])

            o_pass(3, wo_tiles[3])
    nc.compile()
    return nc


def _get_nc():
    if "nc" not in _cache:
        _cache["nc"] = _build_nc()
    return _cache["nc"]


def _head_perm():
    p = []
    for h in range(H):
        base = h * HD
        p += [base + 2 * j for j in range(HD // 2)]
        p += [base + 2 * j + 1 for j in range(HD // 2)]
    return np.array(p)


def _pack_thin(wT):
    # [2048(k), 2048(d)] -> [H, 128(p), KC, 128(d)]
    return np.ascontiguousarray(
        wT.reshape(KC, 128, H, 128).transpose(2, 1, 0, 3)
    )


def _pack_x(xb):
    # [rows 1024, 2048] -> [2(col-half), 128(p), KC, 512] (contiguous halves)
    xt = xb.T.reshape(KC, 128, 2, 512)
    return np.ascontiguousarray(xt.transpose(2, 1, 0, 3))


def _prep_inputs(inputs):
    import ml_dtypes

    x = np.asarray(inputs["x"], np.float32)
    freqs_cos = np.asarray(inputs["freqs_cos"], np.float32)
    freqs_sin = np.asarray(inputs["freqs_sin"], np.float32)
    mask = np.asarray(inputs["mask"], np.float32)
    wq = np.asarray(inputs["wq"], np.float32)
    bq = np.asarray(inputs["bq"], np.float32)
    wk = np.asarray(inputs["wk"], np.float32)
    bk = np.asarray(inputs["bk"], np.float32)
    wv = np.asarray(inputs["wv"], np.float32)
    bv = np.asarray(inputs["bv"], np.float32)
    wo = np.asarray(inputs["wo"], np.float32)
    start_pos = int(np.asarray(inputs.get("start_pos", 0)))

    perm = _head_perm()
    wq_all = _pack_thin(np.ascontiguousarray(wq[perm].T))  # [H,128,KC,128]
    wk_all = _pack_thin(np.ascontiguousarray(wk[perm].T))
    bq_all = bq[perm].reshape(H, 128)  # [H, 128]
    bk_all = bk[perm].reshape(H, 128)

    wvT = np.ascontiguousarray(wv.T)  # [in 2048, out 2048]
    woT = np.ascontiguousarray(wo.T)  # [d 2048, e 2048]

    cosT = freqs_cos[start_pos:start_pos + S].T.astype(np.float32)  # [64, S]
    sinT = freqs_sin[start_pos:start_pos + S].T.astype(np.float32)
    csk2 = np.ascontiguousarray(np.vstack([cosT, cosT]))
    ssk2 = np.ascontiguousarray(np.vstack([-sinT, sinT]))

    m2 = mask[0, 0]  # [S(q), S(k)] additive
    tri8 = np.stack([
        np.exp(m2[kt * 128:kt * 128 + 128, kt * 128:kt * 128 + 128]).T
        for kt in range(8)
    ], axis=1).astype(np.float32)  # [128(k), 8, 128(q)]
    ones128 = np.ones((128, 128), np.float32)
    # widened tip mask [128(k), 256(q)]: first 128 q-cols all-invalid (zeros),
    # last 128 = diagonal tri block (from the actual mask's last diag block)
    triw = np.concatenate([np.zeros((128, 128), np.float32), tri8[:, 7, :]],
                          axis=1)

    in_maps = []
    for c in range(N_CORES):
        b, g = c // 2, c % 2
        hs = slice(HPC * g, HPC * g + HPC)
        wv_pre = np.ascontiguousarray(
            wvT[:, g * 1024:(g + 1) * 1024]
            .reshape(KC, 128, NP, 256).transpose(2, 1, 0, 3))
        woh = woT[:, g * 1024:(g + 1) * 1024].reshape(KC, 128, EHALF)
        wo_pre = np.ascontiguousarray(np.stack(
            [np.stack([woh[2 * p], woh[2 * p + 1],
                       woh[8 + 2 * p], woh[8 + 2 * p + 1]], axis=1)
             for p in range(NP)], axis=0).astype(ml_dtypes.bfloat16))
        bq_p = np.ascontiguousarray(bq_all[hs].T)[:, :, None]  # [128, HPC, 1]
        bk_p = np.ascontiguousarray(bk_all[hs].T)[:, :, None]
        bv_p = np.ascontiguousarray(np.broadcast_to(
            bv[g * 1024:(g + 1) * 1024].reshape(NP, 256)[None], (128, NP, 256)))
        in_maps.append({
            "x_pre": _pack_x(x[b]),
            "wq_pre": np.ascontiguousarray(wq_all[hs]),
            "wk_pre": np.ascontiguousarray(wk_all[hs]),
            "wv_pre": wv_pre,
            "wo_pre": wo_pre,
            "bq_p": bq_p, "bk_p": bk_p, "bv_p": bv_p,
            "csk2": csk2, "ssk2": ssk2,
            "triw": triw, "ones128": ones128,
        })
    return in_maps


def kernel(**inputs):
    from concourse.bass_utils import run_bass_kernel_spmd

    trace = bool(inputs.pop("_trace", False))
    bo = np.asarray(inputs["bo"], np.float32)
    in_maps = _prep_inputs(inputs)

    nc = _get_nc()
    kwargs = {}
    if trace:
        kwargs = {"trace": True, "trace_cores": list(range(N_CORES))}
    res = run_bass_kernel_spmd(nc, in_maps, core_ids=list(range(N_CORES)), **kwargs)
    _cache["last_result"] = res

    out = np.empty((B, S, DIM), np.float32)
    for c in range(N_CORES):
        b, g = c // 2, c % 2
        out[b, :, g * 1024:(g + 1) * 1024] = (
            res.results[c]["y"] + bo[None, g * 1024:(g + 1) * 1024])
    return out



# revision 1
# speedup vs baseline: 1.0092x; 1.0092x over previous
"""Multi-head causal attention (LLaMA RoPE), head-parallel sharding on 8 trn2 cores.

Core c = (batch b=c//2, head-group g=c%2). Each core computes Q/K/V projections
and attention for its 8 heads over the FULL 1024 rows of its batch (zero
duplication), writes its attention output (ot, [8*128 d, 1024 q]) to DRAM, and
a pair AllGather ([0,1],[2,3],...) exchanges ot between the two head-groups of
each batch. Each core then computes the output projection for its e-column
half (full 2048-dim contraction) -> y [1024, 1024].

QKV/attention matmuls fp32r; the gathered ot and wo run bf16 (halves exchange
wire time + wo DMA; ~0.3% rms, well inside the 2e-2 gate). Scores are computed
transposed ST[k,q]; softmax skips max-subtraction; denominator via ones-matmul
accumulated alongside PV; causal mask applied post-exp as a binary multiply on
diagonal 128x128 tiles (exact for the additive -1e9/0 causal mask).
"""

import math
import sys

import numpy as np

sys.path.insert(0, "/opt/trn_rl_repo")

B, S, DIM, H = 4, 1024, 2048, 16
HD = DIM // H  # 128
KC = DIM // 128  # 16 contraction chunks
HPC = H // 2  # 8 heads per core
NP = HPC // 2  # 4 head-pairs per core
EHALF = DIM // 2  # 1024 output cols per core
SCALE = 1.0 / math.sqrt(HD)
N_CORES = 8
PAIRS = [[0, 1], [2, 3], [4, 5], [6, 7]]
# O-proj accumulation order: pair-3 head chunks last (its AllGather lands last)
DC_ORDER = [0, 1, 8, 9, 2, 3, 10, 11, 4, 5, 12, 13, 6, 7, 14, 15]

_cache = {}


def _build_nc():
    import concourse.mybir as mybir
    import concourse.tile as tile
    from concourse import bacc

    F32R = mybir.dt.float32r
    F32 = mybir.dt.float32
    BF16 = mybir.dt.bfloat16

    nc = bacc.Bacc("TRN2", target_bir_lowering=False, debug=False,
                   num_devices=N_CORES)

    x_in = nc.dram_tensor("x_pre", [2, 128, KC, 512], F32R, kind="ExternalInput")
    wq_in = nc.dram_tensor("wq_pre", [HPC, 128, KC, 128], F32R, kind="ExternalInput")
    wk_in = nc.dram_tensor("wk_pre", [HPC, 128, KC, 128], F32R, kind="ExternalInput")
    wv_in = nc.dram_tensor("wv_pre", [NP, 128, KC, 256], F32R, kind="ExternalInput")
    wo_in = nc.dram_tensor("wo_pre", [NP, 128, 4, EHALF], BF16, kind="ExternalInput")
    bq_in = nc.dram_tensor("bq_p", [128, HPC, 1], F32, kind="ExternalInput")
    bk_in = nc.dram_tensor("bk_p", [128, HPC, 1], F32, kind="ExternalInput")
    bv_in = nc.dram_tensor("bv_p", [128, NP, 256], F32, kind="ExternalInput")
    csk_in = nc.dram_tensor("csk2", [128, S], F32R, kind="ExternalInput")
    ssk_in = nc.dram_tensor("ssk2", [128, S], F32R, kind="ExternalInput")
    triw_in = nc.dram_tensor("triw", [128, 256], F32R, kind="ExternalInput")
    ones_in = nc.dram_tensor("ones128", [128, 128], F32R, kind="ExternalInput")
    y_out = nc.dram_tensor("y", [S, EHALF], F32, kind="ExternalOutput")

    with tile.TileContext(nc) as tc:
        with (
            tc.tile_pool(name="consts", bufs=1) as consts,
            tc.tile_pool(name="xpool", bufs=1) as xpool,
            tc.tile_pool(name="trig", bufs=1) as trig,
            tc.tile_pool(name="wpool", bufs=1) as wpool,
            tc.tile_pool(name="qkv", bufs=1) as qkv,
            tc.tile_pool(name="rope", bufs=3) as rope,
            tc.tile_pool(name="ptp", bufs=2) as ptp,
            tc.tile_pool(name="otst", bufs=2) as otst,
            tc.tile_pool(name="sotp", bufs=3) as sotp,
            tc.tile_pool(name="yap", bufs=1) as yap,
            tc.tile_pool(name="dram", bufs=1, space="DRAM") as dram,
            tc.tile_pool(name="ps_qk", bufs=2, space="PSUM") as ps_qk,
            tc.tile_pool(name="ps_v", bufs=1, space="PSUM") as ps_v,
            tc.tile_pool(name="ps_st", bufs=2, space="PSUM") as ps_st,
            tc.tile_pool(name="ps_acc", bufs=3, space="PSUM") as ps_acc,
        ):
            def load_pair_weights(p, x_interleave=None):
                """Allocate + DMA weight tiles for pair p. x_interleave: list of
                (sbuf_dst, dram_src) x-chunk DMAs dripped between weight DMAs
                so the first Q-projection's deps land early."""
                xi = list(x_interleave or [])

                def drip(n):
                    for _ in range(n):
                        if xi:
                            dst, src = xi.pop(0)
                            nc.sync.dma_start(dst, src)

                wq_sb = [wpool.tile([128, KC, 128], F32R, tag=f"wq{lh}",
                                    name="wq_sb") for lh in range(2)]
                wk_sb = [wpool.tile([128, KC, 128], F32R, tag=f"wk{lh}",
                                    name="wk_sb") for lh in range(2)]
                wv_sb = wpool.tile([128, KC, 256], F32R, tag="wv", name="wv_sb")
                nc.sync.dma_start(wq_sb[0][:], wq_in[2 * p])
                drip(8)
                nc.sync.dma_start(wq_sb[1][:], wq_in[2 * p + 1])
                nc.sync.dma_start(wk_sb[0][:], wk_in[2 * p])
                nc.sync.dma_start(wk_sb[1][:], wk_in[2 * p + 1])
                nc.sync.dma_start(wv_sb[:], wv_in[p])
                return wq_sb, wk_sb, wv_sb

            def load_wo(p):
                wo_sb = wpool.tile([128, 4, EHALF], BF16, tag="wo", bufs=2,
                                   name="wo_sb")
                nc.sync.dma_start(wo_sb[:], wo_in[p])
                return wo_sb

            # small rope tables + biases first (needed ~20us in)
            bq_sb = consts.tile([128, HPC, 1], F32, tag="bq")
            bk_sb = consts.tile([128, HPC, 1], F32, tag="bk")
            nc.sync.dma_start(bq_sb[:], bq_in[:])
            nc.sync.dma_start(bk_sb[:], bk_in[:])
            # pair-0 weights interleaved with x chunks so the first
            # Q-projection (wq0 + x) can start early
            x_sb = xpool.tile([128, KC, S], F32R, tag="x", name="x_sb")
            x_chunks = [(x_sb[:, 4 * i:4 * i + 4, h * 512:h * 512 + 512],
                         x_in[h][:, 4 * i:4 * i + 4, :])
                        for h in range(2) for i in range(4)]
            w0 = load_pair_weights(0, x_interleave=x_chunks)

            csk_sb = trig.tile([128, S], F32R, tag="cs", name="csk_sb")
            ssk_sb = trig.tile([128, S], F32R, tag="ss", name="ssk_sb")
            nc.sync.dma_start(csk_sb[:], csk_in[:])
            nc.sync.dma_start(ssk_sb[:], ssk_in[:])

            bv_sb = consts.tile([128, NP, 256], F32, tag="bv")
            nc.sync.dma_start(bv_sb[:], bv_in[:])
            triw_sb = consts.tile([128, 256], F32R)
            nc.sync.dma_start(triw_sb[:], triw_in[:])
            ones_sb = consts.tile([128, 128], F32R)
            nc.sync.dma_start(ones_sb[:], ones_in[:])

            y_acc = yap.tile([128, 8, 2, 512], BF16, name="y_acc")
            wo_tiles = {}
            ag_ins = []
            ag_outs = []
            for p in range(NP - 1):
                ag_ins.append(dram.tile([256, S], BF16, name=f"agi{p}"))
                ag_outs.append(dram.tile([4, 128, S], BF16, name=f"ago{p}"))
            ag3_ins = [dram.tile([256, 512], BF16, name=f"agi3q{qc}")
                       for qc in range(2)]
            ag3_outs = [dram.tile([4, 128, 512], BF16, name=f"ago3q{qc}")
                        for qc in range(2)]

            def rope_chunk(pm, b_ap, cs_ap, ss_ap, dst_ap):
                """dst = rope(pm + bias); all [128, 512]."""
                tmp = rope.tile([128, 512], F32R, tag="tmp", name="tmp")
                nc.scalar.activation(
                    tmp[:], pm, mybir.ActivationFunctionType.Identity, bias=b_ap
                )
                tsw = rope.tile([128, 512], F32R, tag="tsw", name="tsw")
                nc.sync.dma_start(tsw[0:64, :], tmp[64:128, :])
                nc.sync.dma_start(tsw[64:128, :], tmp[0:64, :])
                nc.vector.tensor_mul(tmp[:], tmp[:], cs_ap)
                nc.vector.tensor_mul(tsw[:], tsw[:], ss_ap)
                nc.vector.tensor_add(dst_ap, tmp[:], tsw[:])

            def o_pass(p, wo_p):
                """Accumulate pair p's 4 head-chunks into y (bf16 partials)."""
                for qt in range(8):
                    qcols = slice(qt * 128, qt * 128 + 128)
                    sot = sotp.tile([128, 4, 128], BF16, tag="sot", name="sot")
                    if p < 3:
                        src_ap = ag_outs[p].rearrange("a r c -> r a c")[:, :, qcols]
                    else:
                        src_ap = ag3_outs[qt // 4].rearrange("a r c -> r a c")[
                            :, :, (qt % 4) * 128:(qt % 4) * 128 + 128]
                    nc.sync.dma_start(sot[:], src_ap)
                    for eb in range(2):
                        ecols = slice(eb * 512, eb * 512 + 512)
                        y_ps = ps_qk.tile([128, 512], F32, tag="qk", name="y_ps")
                        for a in range(4):
                            nc.tensor.matmul(y_ps[:], sot[:, a, :],
                                             wo_p[:, a, ecols],
                                             start=(a == 0), stop=(a == 3))
                        ya = y_acc[:, qt, eb, :]
                        if p == 0:
                            nc.vector.tensor_copy(ya, y_ps[:])
                        elif p < 3:
                            nc.vector.tensor_add(ya, ya, y_ps[:])
                        else:
                            y_sb = otst.tile([128, 512], F32, tag="y",
                                             name="y_sb")
                            nc.vector.tensor_add(y_sb[:], ya, y_ps[:])
                            nc.sync.dma_start(y_out[qcols, ecols], y_sb[:])

            w_next = w0
            for p in range(NP):
                wq_sb, wk_sb, wv_sb = w_next
                wo_tiles[p] = load_wo(p)

                # ---- Q/K projections + RoPE ----
                q_sb = [qkv.tile([128, S], F32R, tag=f"q{lh}", name="q_sb")
                        for lh in range(2)]
                k_sb = [qkv.tile([128, S], F32R, tag=f"k{lh}", name="k_sb")
                        for lh in range(2)]
                for lh in range(2):
                    h = 2 * p + lh
                    projs = ((q_sb[lh], wq_sb[lh], bq_sb),
                             (k_sb[lh], wk_sb[lh], bk_sb))
                    # lh1: K before Q so attention-lh0's exps never queue
                    # behind the last projection's rope identities on Act
                    for dst, w_sb, b_sb in (projs if lh == 0 else projs[::-1]):
                        for c in range(2):
                            ccols = slice(c * 512, c * 512 + 512)
                            pm = ps_qk.tile([128, 512], F32, tag="qk", name="pm")
                            for kc in range(KC):
                                nc.tensor.matmul(pm[:], w_sb[:, kc, :],
                                                 x_sb[:, kc, ccols],
                                                 start=(kc == 0),
                                                 stop=(kc == KC - 1))
                            rope_chunk(pm[:], b_sb[:, h, :], csk_sb[:, ccols],
                                       ssk_sb[:, ccols], dst[:, ccols])

                # ---- V projection ----
                v_sb = qkv.tile([128, 8, 256], F32R, tag="v", name="v_sb")
                for st in range(8):
                    scols = slice(st * 128, st * 128 + 128)
                    vp = ps_v.tile([128, 256], F32, tag="v", name="vp")
                    for kc in range(KC):
                        nc.tensor.matmul(vp[:], x_sb[:, kc, scols],
                                         wv_sb[:, kc, :],
                                         start=(kc == 0), stop=(kc == KC - 1))
                    nc.vector.tensor_add(v_sb[:, st, :], vp[:], bv_sb[:, p, :])

                if p + 1 < NP:
                    w_next = load_pair_weights(p + 1)
                if p == 2:
                    o_pass(0, wo_tiles[0])

                # ---- attention, both heads ----
                # pair 3 runs qc-major so its exchange splits into two
                # half-AllGathers, the first issued mid-attention
                lq = [(lh, qc) for lh in range(2) for qc in range(2)] \
                    if p < 3 else [(lh, qc) for qc in range(2) for lh in range(2)]
                for lh, qc in lq:
                    if True:
                        vcols = slice(lh * 128, lh * 128 + 128)
                        kts = range(4) if qc == 0 else range(8)
                        l_ps = ps_acc.tile([128, 512], F32, tag="acc", name="l_ps")
                        o_ps = ps_acc.tile([128, 512], F32, tag="acc", name="o_ps")
                        for kt in kts:
                            if qc == 0:
                                qv = slice(kt * 128, 512)
                                dg = slice(kt * 128, kt * 128 + 128)
                            else:
                                qv = slice((kt - 4) * 128, 512) if kt >= 4 \
                                    else slice(0, 512)
                                dg = slice((kt - 4) * 128, (kt - 4) * 128 + 128) \
                                    if kt >= 4 else None
                            wide = qv.stop - qv.start == 128
                            if wide:
                                qv = slice(qv.start - 128, 512)
                                dg = qv
                            qg = slice(qc * 512 + qv.start, qc * 512 + 512)
                            st_ps = ps_st.tile([128, 512], F32, tag="st",
                                               name="st_ps")
                            nc.tensor.matmul(st_ps[:, qv],
                                             k_sb[lh][:, kt * 128:kt * 128 + 128],
                                             q_sb[lh][:, qg],
                                             start=True, stop=True)
                            pt = ptp.tile([128, 512], F32R, tag="pt", name="pt")
                            nc.scalar.activation(pt[:, qv], st_ps[:, qv],
                                                 mybir.ActivationFunctionType.Exp,
                                                 scale=SCALE)
                            if dg is not None:
                                msk = triw_sb[:] if wide \
                                    else triw_sb[:, 128:256]
                                nc.vector.tensor_mul(pt[:, dg], pt[:, dg], msk)
                            first, last = (kt == kts[0]), (kt == kts[-1])
                            nc.tensor.matmul(l_ps[:, qv], ones_sb[:], pt[:, qv],
                                             start=first, stop=last)
                            nc.tensor.matmul(o_ps[:, qv], v_sb[:, kt, vcols],
                                             pt[:, qv], start=first, stop=last)
                        rl = rope.tile([128, 512], F32, tag="tmp", name="rl")
                        nc.vector.reciprocal_approx_fast(rl[:], l_ps[:])
                        ots = otst.tile([128, 512], BF16, tag="ot", name="ots")
                        nc.vector.tensor_mul(ots[:], o_ps[:], rl[:])
                        if p < 3:
                            nc.sync.dma_start(
                                ag_ins[p][lh * 128:lh * 128 + 128,
                                          qc * 512:qc * 512 + 512], ots[:])
                        else:
                            nc.sync.dma_start(
                                ag3_ins[qc][lh * 128:lh * 128 + 128, :], ots[:])
                            if lh == 1:
                                nc.gpsimd.collective_compute(
                                    "AllGather", mybir.AluOpType.bypass,
                                    replica_groups=PAIRS,
                                    ins=[ag3_ins[qc].opt()],
                                    outs=[ag3_outs[qc].opt()],
                                )

                if p < 3:
                    nc.gpsimd.collective_compute(
                        "AllGather", mybir.AluOpType.bypass, replica_groups=PAIRS,
                        ins=[ag_ins[p].opt()], outs=[ag_outs[p].opt()],
                    )
                if p >= 2:
                    o_pass(p - 1, wo_tiles[p - 1])

            o_pass(3, wo_tiles[3])
    nc.compile()
    return nc


def _get_nc():
    if "nc" not in _cache:
        _cache["nc"] = _build_nc()
    return _cache["nc"]


def _head_perm():
    p = []
    for h in range(H):
        base = h * HD
        p += [base + 2 * j for j in range(HD // 2)]
        p += [base + 2 * j + 1 for j in range(HD // 2)]
    return np.array(p)


def _pack_thin(wT):
    # [2048(k), 2048(d)] -> [H, 128(p), KC, 128(d)]
    return np.ascontiguousarray(
        wT.reshape(KC, 128, H, 128).transpose(2, 1, 0, 3)
    )


def _pack_x(xb):
    # [rows 1024, 2048] -> [2(col-half), 128(p), KC, 512] (contiguous halves)
    xt = xb.T.reshape(KC, 128, 2, 512)
    return np.ascontiguousarray(xt.transpose(2, 1, 0, 3))


def _prep_inputs(inputs):
    import ml_dtypes

    x = np.asarray(inputs["x"], np.float32)
    freqs_cos = np.asarray(inputs["freqs_cos"], np.float32)
    freqs_sin = np.asarray(inputs["freqs_sin"], np.float32)
    mask = np.asarray(inputs["mask"], np.float32)
    wq = np.asarray(inputs["wq"], np.float32)
    bq = np.asarray(inputs["bq"], np.float32)
    wk = np.asarray(inputs["wk"], np.float32)
    bk = np.asarray(inputs["bk"], np.float32)
    wv = np.asarray(inputs["wv"], np.float32)
    bv = np.asarray(inputs["bv"], np.float32)
    wo = np.asarray(inputs["wo"], np.float32)
    start_pos = int(np.asarray(inputs.get("start_pos", 0)))

    perm = _head_perm()
    wq_all = _pack_thin(np.ascontiguousarray(wq[perm].T))  # [H,128,KC,128]
    wk_all = _pack_thin(np.ascontiguousarray(wk[perm].T))
    bq_all = bq[perm].reshape(H, 128)  # [H, 128]
    bk_all = bk[perm].reshape(H, 128)

    wvT = np.ascontiguousarray(wv.T)  # [in 2048, out 2048]
    woT = np.ascontiguousarray(wo.T)  # [d 2048, e 2048]

    cosT = freqs_cos[start_pos:start_pos + S].T.astype(np.float32)  # [64, S]
    sinT = freqs_sin[start_pos:start_pos + S].T.astype(np.float32)
    csk2 = np.ascontiguousarray(np.vstack([cosT, cosT]))
    ssk2 = np.ascontiguousarray(np.vstack([-sinT, sinT]))

    m2 = mask[0, 0]  # [S(q), S(k)] additive
    tri8 = np.stack([
        np.exp(m2[kt * 128:kt * 128 + 128, kt * 128:kt * 128 + 128]).T
        for kt in range(8)
    ], axis=1).astype(np.float32)  # [128(k), 8, 128(q)]
    ones128 = np.ones((128, 128), np.float32)
    # widened tip mask [128(k), 256(q)]: first 128 q-cols all-invalid (zeros),
    # last 128 = diagonal tri block (from the actual mask's last diag block)
    triw = np.concatenate([np.zeros((128, 128), np.float32), tri8[:, 7, :]],
                          axis=1)

    in_maps = []
    for c in range(N_CORES):
        b, g = c // 2, c % 2
        hs = slice(HPC * g, HPC * g + HPC)
        wv_pre = np.ascontiguousarray(
            wvT[:, g * 1024:(g + 1) * 1024]
            .reshape(KC, 128, NP, 256).transpose(2, 1, 0, 3))
        woh = woT[:, g * 1024:(g + 1) * 1024].reshape(KC, 128, EHALF)
        wo_pre = np.ascontiguousarray(np.stack(
            [np.stack([woh[2 * p], woh[2 * p + 1],
                       woh[8 + 2 * p], woh[8 + 2 * p + 1]], axis=1)
             for p in range(NP)], axis=0).astype(ml_dtypes.bfloat16))
        bq_p = np.ascontiguousarray(bq_all[hs].T)[:, :, None]  # [128, HPC, 1]
        bk_p = np.ascontiguousarray(bk_all[hs].T)[:, :, None]
        bv_p = np.ascontiguousarray(np.broadcast_to(
            bv[g * 1024:(g + 1) * 1024].reshape(NP, 256)[None], (128, NP, 256)))
        in_maps.append({
            "x_pre": _pack_x(x[b]),
            "wq_pre": np.ascontiguousarray(wq_all[hs]),
            "wk_pre": np.ascontiguousarray(wk_all[hs]),
            "wv_pre": wv_pre,
            "wo_pre": wo_pre,
            "bq_p": bq_p, "bk_p": bk_p, "bv_p": bv_p,
            "csk2": csk2, "ssk2": ssk2,
            "triw": triw, "ones128": ones128,
        })
    return in_maps


def kernel(**inputs):
    from concourse.bass_utils import run_bass_kernel_spmd

    trace = bool(inputs.pop("_trace", False))
    bo = np.asarray(inputs["bo"], np.float32)
    in_maps = _prep_inputs(inputs)

    nc = _get_nc()
    kwargs = {}
    if trace:
        kwargs = {"trace": True, "trace_cores": list(range(N_CORES))}
    res = run_bass_kernel_spmd(nc, in_maps, core_ids=list(range(N_CORES)), **kwargs)
    _cache["last_result"] = res

    out = np.empty((B, S, DIM), np.float32)
    for c in range(N_CORES):
        b, g = c // 2, c % 2
        out[b, :, g * 1024:(g + 1) * 1024] = (
            res.results[c]["y"] + bo[None, g * 1024:(g + 1) * 1024])
    return out

